# revision 25
# baseline (speedup 1.0000x reference)
"""MoE top-1 routing kernel for Trainium2 (8 NeuronCores).

Math (matches the reference):
    logits = x @ gate_w + gate_b            # [N, E]
    assign = argmax(logits, -1)             # top-1 expert per token
    out[t] = relu(x[t] @ w1[e] + b1[e]) @ w2[e] + b2[e]   where e = assign[t]

The gate is a tiny (4096x1024x8) matmul computed on the host in float64 (the
smallest top1-top2 logit gap in this regime is ~2e-4, orders of magnitude
above fp32 rounding, so the argmax is unambiguous). Tokens are grouped by
expert and dispatched to the cores holding that expert's weights; outputs are
scattered back to token order on the host.

Device sharding: 2-way tensor-parallel expert pairs. Experts are paired
large-count-with-small-count; the two cores of a pair each hold HALF of the
DFF dimension of BOTH experts (same 32MB weight traffic per core as plain
expert-parallel) and process all the pair's tokens through their DFF half.
relu is elementwise so layer 1 halves are independent; layer 2 produces
partial sums over the DFF half which the host adds. This balances tokens per
core and keeps every matmul's moving dim large (fewer weight reloads -> the
TensorE LDWEIGHTS stream stays hidden under the matmuls).

Per-core device kernel (bf16 matmul datapath, fp32 PSUM accumulation):
    layer1: hT[m*128+p, c] = relu(sum_k w1h[k,: x m,:]^T @ xT[k,: x c] + b1h)
    layer2: yT[m2*128+p, c] = sum_k2 w2h[k2,: x m2,:]^T @ hT[k2,: x c]
Contraction stays on SBUF partitions, tokens on the free dim: no on-device
transposes. The host pre-tiles weights so every DMA is contiguous.

bf16 vs fp32 datapath: TensorE streams 1 row/cycle for both bf16 and
float32r, so compute time is unchanged, but the kernel was DMA-bound
(38MB of fp32 weights at ~250GB/s had the DMA engines 98% busy and
TensorE stalling on weight arrival). bf16 halves the weight/activation
traffic. Accumulation stays fp32 in PSUM; measured rel err ~1e-3, well
inside the 2e-2 gate.
"""

import numpy as np
import ml_dtypes

BF16 = np.dtype(ml_dtypes.bfloat16)

N_TOK, D, DFF, E = 4096, 1024, 4096, 8
P = 128
KD = D // P  # 8 contraction chunks of the d dimension
MH = (DFF // 2) // P  # 16 dff-half blocks (layer1 out / layer2 contraction)

# test.py hooks: set TRACE=True (after installing the NTFF hook) to profile.
TRACE = False
TRACE_CORES = None
LAST_RESULT = None

_PROGRAM_CACHE = {}


def _pad_cap(n):
    """Token capacity: multiple of 8, >=256 (float32r needs moving dim >=256
    for full rate)."""
    return max(256, -(-n // 8) * 8)


def _chunk_sizes(C):
    """Split C tokens into moving-dim chunks <=512 (PSUM bank), balanced so
    every chunk stays >=256 when C >= 512."""
    n = -(-C // 512)
    base, rem = divmod(C, n)
    return [base + (1 if i < rem else 0) for i in range(n)]


def _build_program_tp2(C1, C2):
    import concourse.mybir as mybir
    import concourse.tile as tile
    from concourse import bacc

    f32 = mybir.dt.float32
    bf16 = mybir.dt.bfloat16
    AF = mybir.ActivationFunctionType

    chunks1 = _chunk_sizes(C1)
    chunks2 = _chunk_sizes(C2)

    nc = bacc.Bacc("TRN2", target_bir_lowering=False, debug=False, num_devices=E)

    xt1_d = nc.dram_tensor("xt1", [P, KD * C1], bf16, kind="ExternalInput").ap()
    xt2_d = nc.dram_tensor("xt2", [P, KD * C2], bf16, kind="ExternalInput").ap()
    w1a_d = nc.dram_tensor("w1a", [MH, P, D], bf16, kind="ExternalInput").ap()
    w1b_d = nc.dram_tensor("w1b", [MH, P, D], bf16, kind="ExternalInput").ap()
    b1a_d = nc.dram_tensor("b1a", [P, MH], f32, kind="ExternalInput").ap()
    b1b_d = nc.dram_tensor("b1b", [P, MH], f32, kind="ExternalInput").ap()
    w2a_d = nc.dram_tensor("w2a", [KD, P, MH * P], bf16, kind="ExternalInput").ap()
    w2b_d = nc.dram_tensor("w2b", [KD, P, MH * P], bf16, kind="ExternalInput").ap()
    yt1_d = nc.dram_tensor("yt1", [KD, P, C1], bf16, kind="ExternalOutput").ap()
    yt2_d = nc.dram_tensor("yt2", [KD, P, C2], bf16, kind="ExternalOutput").ap()

    with tile.TileContext(nc) as tc:
        with (
            tc.tile_pool(name="xt_pool", bufs=1) as xt_pool,
            tc.tile_pool(name="ht_pool", bufs=1) as ht_pool,
            # deep weight prefetch: with bufs=4 the Sync engine sat blocked on
            # buffer-free semaphores (trace: DMA_DIRECT2D evt_wait up to 46us)
            # and the weight stream ran just behind LDWEIGHTS all run long
            tc.tile_pool(name="w1_pool", bufs=20) as w1_pool,
            tc.tile_pool(name="w2_pool", bufs=10) as w2_pool,
            tc.tile_pool(name="y_pool", bufs=4) as y_pool,
            tc.tile_pool(name="bias_pool", bufs=1) as bias_pool,
            tc.tile_pool(name="psum", bufs=8, space="PSUM") as psum_pool,
        ):
            # PE p-state warm-up: the tensor clock ramps 0.65->1.2->2.4GHz
            # with ~3us of sustained activity. Burn the dead DMA-latency
            # window (first weights land ~9.5us after the ~7us framework
            # preamble) on dummy matmuls over a zeroed tile so the first real
            # matmuls run closer to full clock.
            warm_sb = bias_pool.tile([P, 256], bf16)
            nc.vector.memset(warm_sb[:], 0.0)
            warm_ps = psum_pool.tile([P, 256], f32, tag="ps")
            for i in range(5):
                nc.tensor.matmul(
                    warm_ps[:],
                    lhsT=warm_sb[:, :P],
                    rhs=warm_sb[:],
                    start=(i == 0),
                    stop=(i == 4),
                )

            xt1_sb = xt_pool.tile([P, KD * C1], bf16)
            xt2_sb = xt_pool.tile([P, KD * C2], bf16)
            # ALL inputs ride the single Sync HWDGE queue, ordered by first
            # consumption: a FIFO queue drains at full aggregate DMA bandwidth
            # and in-order completion is exact prioritization. (Splitting the
            # early loads across gpsimd/scalar queues let the deep w1a
            # prefetch starve the critical xt1 slices.)
            w1_first = w1_pool.tile([P, D], bf16, tag="w1")
            nc.sync.dma_start(w1_first[:], w1a_d[0])
            nc.sync.dma_start(xt1_sb[:, :C1], xt1_d[:, :C1])
            nc.sync.dma_start(xt1_sb[:, C1 : 4 * C1], xt1_d[:, C1 : 4 * C1])
            nc.sync.dma_start(xt1_sb[:, 4 * C1 :], xt1_d[:, 4 * C1 :])
            b1a_sb = bias_pool.tile([P, MH], f32)
            nc.sync.dma_start(b1a_sb[:], b1a_d[:])
            b1b_sb = bias_pool.tile([P, MH], f32)
            nc.sync.dma_start(b1b_sb[:], b1b_d[:])

            ht1_sb = ht_pool.tile([P, MH * C1], bf16)
            ht2_sb = ht_pool.tile([P, MH * C2], bf16)

            def layer1(m, w1_sb, C, chunks, xt_sb, ht_sb, b1_sb):
                t0 = 0
                for tn in chunks:
                    ps = psum_pool.tile([P, 512], f32, tag="ps")
                    for k in range(KD):
                        nc.tensor.matmul(
                            ps[:, :tn],
                            lhsT=w1_sb[:, k * P : (k + 1) * P],
                            rhs=xt_sb[:, k * C + t0 : k * C + t0 + tn],
                            start=(k == 0),
                            stop=(k == KD - 1),
                        )
                    nc.scalar.activation(
                        ht_sb[:, m * C + t0 : m * C + t0 + tn],
                        ps[:, :tn],
                        AF.Relu,
                        bias=b1_sb[:, m : m + 1],
                    )
                    t0 += tn

            def layer2(m2, w2_sb, C, chunks, ht_sb, yt_d, last=False):
                t0 = 0
                for tn in chunks:
                    ps2 = psum_pool.tile([P, 512], f32, tag="ps")
                    for k2 in range(MH):
                        nc.tensor.matmul(
                            ps2[:, :tn],
                            lhsT=w2_sb[:, k2 * P : (k2 + 1) * P],
                            rhs=ht_sb[:, k2 * C + t0 : k2 * C + t0 + tn],
                            start=(k2 == 0),
                            stop=(k2 == MH - 1),
                        )
                    yt_sb = y_pool.tile([P, 512], bf16, tag="yt")
                    # partial sum over this core's DFF half; b2 added on host
                    if last and t0 + tn == C:
                        # kernel-final block is on the critical path after the
                        # last matmul: split activation+store in halves across
                        # the two HWDGE queues (sync + scalar) so the two
                        # store latencies overlap; gpsimd SWDGE is ~1us slower
                        h1 = tn // 2
                        nc.scalar.activation(yt_sb[:, :h1], ps2[:, :h1], AF.Identity)
                        nc.sync.dma_start(yt_d[m2][:, t0 : t0 + h1], yt_sb[:, :h1])
                        nc.scalar.activation(yt_sb[:, h1:tn], ps2[:, h1:tn], AF.Identity)
                        nc.scalar.dma_start(
                            yt_d[m2][:, t0 + h1 : t0 + tn], yt_sb[:, h1:tn]
                        )
                    else:
                        nc.scalar.activation(yt_sb[:, :tn], ps2[:, :tn], AF.Identity)
                        nc.gpsimd.dma_start(yt_d[m2][:, t0 : t0 + tn], yt_sb[:, :tn])
                    t0 += tn

            for m in range(MH):
                if m == 0:
                    w1_sb = w1_first
                else:
                    w1_sb = w1_pool.tile([P, D], bf16, tag="w1")
                    nc.sync.dma_start(w1_sb[:], w1a_d[m])
                if m == 5:
                    # xt2 slots into the Sync stream here: early enough to
                    # land ~25us before L1b consumes it, late enough that it
                    # never delays a w1a block past its consumption time
                    nc.sync.dma_start(xt2_sb[:, : 4 * C2], xt2_d[:, : 4 * C2])
                    nc.sync.dma_start(xt2_sb[:, 4 * C2 :], xt2_d[:, 4 * C2 :])
                layer1(m, w1_sb, C1, chunks1, xt1_sb, ht1_sb, b1a_sb)
            for m in range(MH):
                w1_sb = w1_pool.tile([P, D], bf16, tag="w1")
                nc.sync.dma_start(w1_sb[:], w1b_d[m])
                layer1(m, w1_sb, C2, chunks2, xt2_sb, ht2_sb, b1b_sb)

            for m2 in range(KD):
                w2_sb = w2_pool.tile([P, MH * P], bf16, tag="w2")
                nc.sync.dma_start(w2_sb[:], w2a_d[m2])
                layer2(m2, w2_sb, C1, chunks1, ht1_sb, yt1_d)
            for m2 in range(KD):
                w2_sb = w2_pool.tile([P, MH * P], bf16, tag="w2")
                nc.sync.dma_start(w2_sb[:], w2b_d[m2])
                layer2(m2, w2_sb, C2, chunks2, ht2_sb, yt2_d, last=(m2 == KD - 1))

    nc.compile()
    return nc


MQ = (DFF // 4) // P  # 8 dff-quarter blocks (TP4 layer1 out / layer2 contraction)


def _build_program_tp4(caps):
    """TP4: 4 cores per expert-quad; core holds a DFF/4 slice of 4 experts.

    Near-perfect load balance: per-core rows = 128 * sum(caps) with caps the
    slotwise maxima over the two quads (2088 for this gate vs 2160 for TP2's
    pair scheme) -> ~111.4us ideal row stream vs 115.2. Host sums 4 partials.
    """
    import concourse.mybir as mybir
    import concourse.tile as tile
    from concourse import bacc

    f32 = mybir.dt.float32
    bf16 = mybir.dt.bfloat16
    AF = mybir.ActivationFunctionType

    S = list(caps)
    chunks = [_chunk_sizes(c) for c in S]

    nc = bacc.Bacc("TRN2", target_bir_lowering=False, debug=False, num_devices=E)

    # slot 0 is declared [P, KD, S0] so a per-chunk 3D slice ([:, :, t0:t1])
    # is a single DMA: the real stream is gated on w1q[0] + chunk0 only
    # (0.84MB) instead of the full slot-0 token block (1.4MB)
    xt0_d = nc.dram_tensor("xt0", [P, KD, S[0]], bf16, kind="ExternalInput").ap()
    xt_d = [xt0_d] + [
        nc.dram_tensor(f"xt{s}", [P, KD * S[s]], bf16, kind="ExternalInput").ap()
        for s in range(1, 4)
    ]
    w1q_d = nc.dram_tensor("w1q", [4 * MQ, P, D], bf16, kind="ExternalInput").ap()
    b1q_d = nc.dram_tensor("b1q", [P, 4 * MQ], f32, kind="ExternalInput").ap()
    w2q_d = nc.dram_tensor("w2q", [4 * KD, P, MQ * P], bf16, kind="ExternalInput").ap()
    yt_d = [
        nc.dram_tensor(f"yt{s}", [KD, P, S[s]], bf16, kind="ExternalOutput").ap()
        for s in range(4)
    ]

    with tile.TileContext(nc) as tc:
        with (
            tc.tile_pool(name="xt_pool", bufs=1) as xt_pool,
            tc.tile_pool(name="ht_pool", bufs=1) as ht_pool,
            tc.tile_pool(name="w1_pool", bufs=20) as w1_pool,
            tc.tile_pool(name="w2_pool", bufs=12) as w2_pool,
            tc.tile_pool(name="y_pool", bufs=4) as y_pool,
            tc.tile_pool(name="bias_pool", bufs=1) as bias_pool,
            tc.tile_pool(name="psum", bufs=8, space="PSUM") as psum_pool,
        ):
            # PE p-state warm-up AND idle-bridge: the tensor clock ramps
            # 0.65->1.2->2.4GHz over ~3us of sustained activity and the ramp
            # resets on idle gaps. Dummy matmuls keep the PE busy from the end
            # of the framework preamble (~7us) until xt0 is fully resident
            # (~12us), so the real stream starts at full clock with no gap.
            warm_sb = bias_pool.tile([P, 256], bf16)
            nc.vector.memset(warm_sb[:], 0.0)
            warm_ps = psum_pool.tile([P, 256], f32, tag="ps")
            for i in range(22):
                nc.tensor.matmul(
                    warm_ps[:],
                    lhsT=warm_sb[:, :P],
                    rhs=warm_sb[:],
                    start=(i == 0),
                    stop=(i == 21),
                )

            # slot 0 gets one SBUF tile per token chunk (fed by one DMA each);
            # slots 1-3 keep a single flat tile
            xt0c_sb = [
                xt_pool.tile([P, KD * tn], bf16, name=f"xt0c{ci}")
                for ci, tn in enumerate(chunks[0])
            ]
            xt_sb = [None] + [
                xt_pool.tile([P, KD * S[s]], bf16, name=f"xt_sb{s}")
                for s in range(1, 4)
            ]
            ht_sb = [
                ht_pool.tile([P, MQ * S[s]], bf16, name=f"ht_sb{s}") for s in range(4)
            ]

            w1_first = w1_pool.tile([P, D], bf16, tag="w1")
            nc.sync.dma_start(w1_first[:], w1q_d[0])
            t0 = 0
            for ci, tn in enumerate(chunks[0]):
                q = nc.sync if ci == 0 else nc.scalar
                q.dma_start(xt0c_sb[ci][:], xt0_d[:, :, t0 : t0 + tn])
                t0 += tn
            b1q_sb = bias_pool.tile([P, 4 * MQ], f32)
            nc.sync.dma_start(b1q_sb[:], b1q_d[:])

            def l1_chunk(e, m, w1_sb, s, rhs_fn, tn, t0):
                C = S[s]
                ps = psum_pool.tile([P, 512], f32, tag="ps")
                for k in range(KD):
                    nc.tensor.matmul(
                        ps[:, :tn],
                        lhsT=w1_sb[:, k * P : (k + 1) * P],
                        rhs=rhs_fn(k),
                        start=(k == 0),
                        stop=(k == KD - 1),
                    )
                nc.scalar.activation(
                    ht_sb[s][:, m * C + t0 : m * C + t0 + tn],
                    ps[:, :tn],
                    AF.Relu,
                    bias=b1q_sb[:, e * MQ + m : e * MQ + m + 1],
                )

            def layer2(e, m2, w2_sb, s, last=False):
                C, t0 = S[s], 0
                for tn in chunks[s]:
                    ps2 = psum_pool.tile([P, 512], f32, tag="ps")
                    for k2 in range(MQ):
                        nc.tensor.matmul(
                            ps2[:, :tn],
                            lhsT=w2_sb[:, k2 * P : (k2 + 1) * P],
                            rhs=ht_sb[s][:, k2 * C + t0 : k2 * C + t0 + tn],
                            start=(k2 == 0),
                            stop=(k2 == MQ - 1),
                        )
                    yt_sb = y_pool.tile([P, 512], bf16, tag="yt")
                    if last and t0 + tn == C:
                        h1 = tn // 2
                        nc.scalar.activation(yt_sb[:, :h1], ps2[:, :h1], AF.Identity)
                        nc.sync.dma_start(yt_d[s][m2][:, t0 : t0 + h1], yt_sb[:, :h1])
                        nc.scalar.activation(yt_sb[:, h1:tn], ps2[:, h1:tn], AF.Identity)
                        nc.scalar.dma_start(
                            yt_d[s][m2][:, t0 + h1 : t0 + tn], yt_sb[:, h1:tn]
                        )
                    else:
                        nc.scalar.activation(yt_sb[:, :tn], ps2[:, :tn], AF.Identity)
                        nc.gpsimd.dma_start(
                            yt_d[s][m2][:, t0 : t0 + tn], yt_sb[:, :tn]
                        )
                    t0 += tn

            def load_xt(s):
                nc.sync.dma_start(xt_sb[s][:, : 4 * S[s]], xt_d[s][:, : 4 * S[s]])
                nc.sync.dma_start(xt_sb[s][:, 4 * S[s] :], xt_d[s][:, 4 * S[s] :])

            # slot 0 runs chunk-outer: all MQ m-blocks over chunk 0 first, so
            # the stream starts once w1q[0]+chunk0 (0.84MB) land; the w1 tiles
            # stay in the pool and the chunk-1 pass reuses them with zero DMA
            # demand — that dead window streams the other slots' tokens in
            w1_s0 = []
            nc0 = len(chunks[0])
            xt1_at = MQ + 1 if nc0 > 1 else 4
            xt2_at = MQ + 5 if nc0 > 1 else 6
            t0 = 0
            it = 0
            for ci, tn in enumerate(chunks[0]):
                for m in range(MQ):
                    if ci == 0:
                        if m == 0:
                            w1_sb = w1_first
                        else:
                            w1_sb = w1_pool.tile([P, D], bf16, tag="w1")
                            nc.sync.dma_start(w1_sb[:], w1q_d[m])
                        w1_s0.append(w1_sb)
                    if it == xt1_at:
                        load_xt(1)
                    if it == xt2_at:
                        load_xt(2)
                    it += 1
                    xc = xt0c_sb[ci]
                    l1_chunk(
                        0, m, w1_s0[m], 0,
                        lambda k, xc=xc, tn=tn: xc[:, k * tn : (k + 1) * tn],
                        tn, t0,
                    )
                t0 += tn

            for e in range(1, 4):
                for m in range(MQ):
                    w1_sb = w1_pool.tile([P, D], bf16, tag="w1")
                    nc.sync.dma_start(w1_sb[:], w1q_d[e * MQ + m])
                    if e == 1 and m == 1:
                        load_xt(3)
                    C, te = S[e], 0
                    for tn in chunks[e]:
                        xe = xt_sb[e]
                        l1_chunk(
                            e, m, w1_sb, e,
                            lambda k, xe=xe, C=C, te=te, tn=tn: xe[
                                :, k * C + te : k * C + te + tn
                            ],
                            tn, te,
                        )
                        te += tn

            for e in range(4):
                for m2 in range(KD):
                    w2_sb = w2_pool.tile([P, MQ * P], bf16, tag="w2")
                    nc.sync.dma_start(w2_sb[:], w2q_d[e * KD + m2])
                    layer2(e, m2, w2_sb, e, last=(e == 3 and m2 == KD - 1))

    nc.compile()
    return nc


def _arrange_w1_quarter(w1_e, h):
    """w1 quarter: [D, 1024] -> [MQ, P, D] with [m,p,k*128+j] = w1[k*128+p, off+m*128+j]."""
    q = w1_e[:, h * (MQ * P) : (h + 1) * (MQ * P)].astype(BF16)
    return np.ascontiguousarray(
        q.reshape(KD, P, MQ, P).transpose(2, 1, 0, 3).reshape(MQ, P, D)
    )


def _arrange_w2_quarter(w2_e, h):
    """w2 quarter: [1024, D] -> [KD, P, MQ*P] with [m2,p,k2*128+j] = w2[off+k2*128+p, m2*128+j]."""
    q = w2_e[h * (MQ * P) : (h + 1) * (MQ * P), :].astype(BF16)
    return np.ascontiguousarray(
        q.reshape(MQ, P, KD, P).transpose(2, 1, 0, 3).reshape(KD, P, MQ * P)
    )


def _run_pass_tp4(x, w1, b1, w2, b2, idx, out):
    from concourse.bass_utils import run_bass_kernel_spmd

    global LAST_RESULT

    counts = np.array([len(i) for i in idx])
    order = np.argsort(-counts, kind="stable")
    # slot j holds the (2j)-th and (2j+1)-th largest experts, one per quad:
    # slot capacity = the larger of the two, and sum(caps) is minimal
    groups = [[int(order[2 * j + g]) for j in range(4)] for g in range(2)]
    caps = tuple(_pad_cap(max(counts[order[2 * j]], counts[order[2 * j + 1]]))
                 for j in range(4))

    key = ("tp4",) + caps
    if key not in _PROGRAM_CACHE:
        _PROGRAM_CACHE[key] = _build_program_tp4(caps)
    nc = _PROGRAM_CACHE[key]

    in_maps = []
    for c in range(E):
        g, h = divmod(c, 4)
        exps = groups[g]
        m = {
            "w1q": np.concatenate([_arrange_w1_quarter(w1[e], h) for e in exps]),
            "b1q": np.ascontiguousarray(
                np.concatenate(
                    [
                        b1[e][h * (MQ * P) : (h + 1) * (MQ * P)].reshape(MQ, P).T
                        for e in exps
                    ],
                    axis=1,
                )
            ),
            "w2q": np.concatenate([_arrange_w2_quarter(w2[e], h) for e in exps]),
        }
        for s, e in enumerate(exps):
            xt = _arrange_tokens(x[idx[e]], caps[s])
            # slot 0's dram tensor is declared [P, KD, S0] (same bytes)
            m[f"xt{s}"] = xt.reshape(P, KD, caps[s]) if s == 0 else xt
        in_maps.append(m)

    res = run_bass_kernel_spmd(
        nc,
        in_maps,
        core_ids=list(range(E)),
        trace=TRACE,
        **({"trace_cores": TRACE_CORES} if TRACE_CORES else {}),
    )
    LAST_RESULT = res

    for g in range(2):
        for s, e in enumerate(groups[g]):
            n = len(idx[e])
            if n == 0:
                continue
            yt = sum(
                res.results[4 * g + h][f"yt{s}"].astype(np.float32) for h in range(4)
            )
            ye = yt.transpose(2, 0, 1).reshape(-1, D)
            out[idx[e]] = ye[:n] + b2[e]


def _arrange_tokens(x_e, C):
    """[n, D] tokens -> xt[p, k*C + c] = x_e[c, k*128 + p], zero-padded, bf16."""
    xe = np.zeros((C, D), BF16)
    xe[: len(x_e)] = x_e
    return np.ascontiguousarray(
        xe.T.reshape(KD, P, C).transpose(1, 0, 2).reshape(P, KD * C)
    )


def _arrange_w1_half(w1_e, h):
    """w1 half: [D, 2048] -> [MH, P, D] with [m, p, k*128+j] = w1[k*128+p, off+m*128+j]."""
    half = w1_e[:, h * (MH * P) : (h + 1) * (MH * P)].astype(BF16)
    return np.ascontiguousarray(
        half.reshape(KD, P, MH, P).transpose(2, 1, 0, 3).reshape(MH, P, D)
    )


def _arrange_w2_half(w2_e, h):
    """w2 half: [2048, D] -> [KD, P, 2048] with [m2, p, k2*128+j] = w2[off+k2*128+p, m2*128+j]."""
    half = w2_e[h * (MH * P) : (h + 1) * (MH * P), :].astype(BF16)
    return np.ascontiguousarray(
        half.reshape(MH, P, KD, P).transpose(2, 1, 0, 3).reshape(KD, P, MH * P)
    )


def kernel(x, gate_w, gate_b, w1, b1, w2, b2):
    from concourse.bass_utils import run_bass_kernel_spmd

    global LAST_RESULT

    x = np.ascontiguousarray(np.asarray(x, dtype=np.float32))
    gate_w = np.asarray(gate_w, dtype=np.float32)
    gate_b = np.asarray(gate_b, dtype=np.float32)
    w1 = np.asarray(w1, dtype=np.float32)
    b1 = np.asarray(b1, dtype=np.float32)
    w2 = np.asarray(w2, dtype=np.float32)
    b2 = np.asarray(b2, dtype=np.float32)
    n_tok = x.shape[0]

    # host gate + top-1 routing (fp64: exact argmax, see module docstring)
    logits = x.astype(np.float64) @ gate_w.astype(np.float64) + gate_b.astype(
        np.float64
    )
    assign = np.argmax(logits, axis=-1)
    idx_full = [np.nonzero(assign == e)[0] for e in range(E)]

    # Defensive slabbing: if routing were pathologically imbalanced, process
    # tokens in passes so per-expert capacity stays within SBUF limits. With
    # the benchmark's near-uniform gate this is a single pass.
    slab = 960
    n_pass = max(1, -(-max(len(i) for i in idx_full) // slab))
    out = np.zeros((n_tok, D), np.float32)
    for ps in range(n_pass):
        idx = [i[ps * slab : (ps + 1) * slab] for i in idx_full]
        _run_pass(x, w1, b1, w2, b2, idx, out)
    return out


USE_TP4 = True


def _run_pass(x, w1, b1, w2, b2, idx, out):
    from concourse.bass_utils import run_bass_kernel_spmd

    global LAST_RESULT

    if USE_TP4:
        return _run_pass_tp4(x, w1, b1, w2, b2, idx, out)

    counts = np.array([len(i) for i in idx])

    # pair experts large-with-small to balance per-core token load
    order = np.argsort(-counts, kind="stable")
    pairs = [(int(order[p]), int(order[E - 1 - p])) for p in range(E // 2)]
    C1 = _pad_cap(max(counts[a] for a, _ in pairs))
    C2 = _pad_cap(max(counts[b] for _, b in pairs))

    key = (C1, C2)
    if key not in _PROGRAM_CACHE:
        _PROGRAM_CACHE[key] = _build_program_tp2(C1, C2)
    nc = _PROGRAM_CACHE[key]

    in_maps = []
    for c in range(E):
        p, h = divmod(c, 2)
        ea, eb = pairs[p]
        in_maps.append(
            {
                "xt1": _arrange_tokens(x[idx[ea]], C1),
                "xt2": _arrange_tokens(x[idx[eb]], C2),
                "w1a": _arrange_w1_half(w1[ea], h),
                "w1b": _arrange_w1_half(w1[eb], h),
                "b1a": np.ascontiguousarray(
                    b1[ea][h * (MH * P) : (h + 1) * (MH * P)].reshape(MH, P).T
                ),
                "b1b": np.ascontiguousarray(
                    b1[eb][h * (MH * P) : (h + 1) * (MH * P)].reshape(MH, P).T
                ),
                "w2a": _arrange_w2_half(w2[ea], h),
                "w2b": _arrange_w2_half(w2[eb], h),
            }
        )

    res = run_bass_kernel_spmd(
        nc,
        in_maps,
        core_ids=list(range(E)),
        trace=TRACE,
        **({"trace_cores": TRACE_CORES} if TRACE_CORES else {}),
    )
    LAST_RESULT = res

    for p in range(E // 2):
        ea, eb = pairs[p]
        for slot, e in (("yt1", ea), ("yt2", eb)):
            n = len(idx[e])
            if n == 0:
                continue
            # sum the two DFF-half partials (bf16 on the wire, fp32 combine),
            # restore [tokens, D], add b2
            yt = res.results[2 * p][slot].astype(np.float32) + res.results[2 * p + 1][
                slot
            ].astype(np.float32)
            ye = yt.transpose(2, 0, 1).reshape(-1, D)
            out[idx[e]] = ye[:n] + b2[e]



# revision 31
# speedup vs baseline: 1.0155x; 1.0155x over previous
"""MoE top-1 routing kernel for Trainium2 (8 NeuronCores).

Math (matches the reference):
    logits = x @ gate_w + gate_b            # [N, E]
    assign = argmax(logits, -1)             # top-1 expert per token
    out[t] = relu(x[t] @ w1[e] + b1[e]) @ w2[e] + b2[e]   where e = assign[t]

The gate is a tiny (4096x1024x8) matmul computed on the host in float64 (the
smallest top1-top2 logit gap in this regime is ~2e-4, orders of magnitude
above fp32 rounding, so the argmax is unambiguous). Tokens are grouped by
expert and dispatched to the cores holding that expert's weights; outputs are
scattered back to token order on the host.

Device sharding (TP4 expert-quads): experts are split into two quads of 4
(snake-ordered by token count so slotwise maxima are minimal); each quad maps
to 4 cores, and each core holds a DFF/4 slice of all 4 experts in its quad
(16MB bf16 weights per core, same total traffic as expert-parallel). Every
core processes all its quad's tokens through its DFF-quarter; relu is
elementwise so layer-1 slices are independent, and layer 2 produces partial
sums over the quarter which the host adds (4 partials per token). Per-core
matmul rows = 128 * sum(slot capacities) ~ 267k rows -> ~111.4us ideal at
1 row/cycle/2.4GHz, within ~4% of a perfectly balanced 109.2us. (The earlier
pair-based TP2 scheme needed 115.2us ideal: slot capacity is a cross-pair max
so imbalance compounds; quads average it out.)

Per-core device kernel (bf16 matmul datapath, fp32 PSUM accumulation):
    layer1: hT[m*128+p, c] = relu(sum_k w1h[k,: x m,:]^T @ xT[k,: x c] + b1h)
    layer2: yT[m2*128+p, c] = sum_k2 w2h[k2,: x m2,:]^T @ hT[k2,: x c]
Contraction stays on SBUF partitions, tokens on the free dim: no on-device
transposes. The host pre-tiles weights so every DMA is contiguous.

bf16 vs fp32 datapath: TensorE streams 1 row/cycle for both bf16 and
float32r, so compute time is unchanged, but the kernel was DMA-bound
(38MB of fp32 weights at ~250GB/s had the DMA engines 98% busy and
TensorE stalling on weight arrival), and fp32 LDWEIGHTS (~190ns vs
~100ns bf16) serialized short matmuls. bf16 halves the traffic and
hides LDWEIGHTS. Accumulation stays fp32 in PSUM; measured rel err
~3e-3, well inside the 2e-2 gate.

Schedule notes (from NTFF traces): all inputs ride the single Sync HWDGE
queue in consumption order (a FIFO drains at full aggregate bandwidth, so
in-order completion is exact prioritization); weight tile pools are deep
(bufs=20/12) because shallow pools gate DMA issue on buffer-free semaphores
and starve LDWEIGHTS; 28 dummy matmuls bridge the ~7us framework preamble to
the first real matmul so the PE p-state (0.65->1.2->2.4GHz over ~3us busy)
is fully ramped and never resets; the kernel-final store is split across the
sync+scalar HWDGE queues to overlap the two ~2.4us DMA latency chains.
"""

import numpy as np
import ml_dtypes

BF16 = np.dtype(ml_dtypes.bfloat16)

N_TOK, D, DFF, E = 4096, 1024, 4096, 8
P = 128
KD = D // P  # 8 contraction chunks of the d dimension
MH = (DFF // 2) // P  # 16 dff-half blocks (layer1 out / layer2 contraction)

# test.py hooks: set TRACE=True (after installing the NTFF hook) to profile.
TRACE = False
TRACE_CORES = None
LAST_RESULT = None

_PROGRAM_CACHE = {}


def _pad_cap(n):
    """Token capacity: multiple of 8, >=256 (float32r needs moving dim >=256
    for full rate)."""
    return max(256, -(-n // 8) * 8)


def _chunk_sizes(C):
    """Split C tokens into moving-dim chunks <=512 (PSUM bank), balanced so
    every chunk stays >=256 when C >= 512."""
    n = -(-C // 512)
    base, rem = divmod(C, n)
    return [base + (1 if i < rem else 0) for i in range(n)]


def _build_program_tp2(C1, C2):
    import concourse.mybir as mybir
    import concourse.tile as tile
    from concourse import bacc

    f32 = mybir.dt.float32
    bf16 = mybir.dt.bfloat16
    AF = mybir.ActivationFunctionType

    chunks1 = _chunk_sizes(C1)
    chunks2 = _chunk_sizes(C2)

    nc = bacc.Bacc("TRN2", target_bir_lowering=False, debug=False, num_devices=E)

    xt1_d = nc.dram_tensor("xt1", [P, KD * C1], bf16, kind="ExternalInput").ap()
    xt2_d = nc.dram_tensor("xt2", [P, KD * C2], bf16, kind="ExternalInput").ap()
    w1a_d = nc.dram_tensor("w1a", [MH, P, D], bf16, kind="ExternalInput").ap()
    w1b_d = nc.dram_tensor("w1b", [MH, P, D], bf16, kind="ExternalInput").ap()
    b1a_d = nc.dram_tensor("b1a", [P, MH], f32, kind="ExternalInput").ap()
    b1b_d = nc.dram_tensor("b1b", [P, MH], f32, kind="ExternalInput").ap()
    w2a_d = nc.dram_tensor("w2a", [KD, P, MH * P], bf16, kind="ExternalInput").ap()
    w2b_d = nc.dram_tensor("w2b", [KD, P, MH * P], bf16, kind="ExternalInput").ap()
    yt1_d = nc.dram_tensor("yt1", [KD, P, C1], bf16, kind="ExternalOutput").ap()
    yt2_d = nc.dram_tensor("yt2", [KD, P, C2], bf16, kind="ExternalOutput").ap()

    with tile.TileContext(nc) as tc:
        with (
            tc.tile_pool(name="xt_pool", bufs=1) as xt_pool,
            tc.tile_pool(name="ht_pool", bufs=1) as ht_pool,
            # deep weight prefetch: with bufs=4 the Sync engine sat blocked on
            # buffer-free semaphores (trace: DMA_DIRECT2D evt_wait up to 46us)
            # and the weight stream ran just behind LDWEIGHTS all run long
            tc.tile_pool(name="w1_pool", bufs=20) as w1_pool,
            tc.tile_pool(name="w2_pool", bufs=10) as w2_pool,
            tc.tile_pool(name="y_pool", bufs=4) as y_pool,
            tc.tile_pool(name="bias_pool", bufs=1) as bias_pool,
            tc.tile_pool(name="psum", bufs=8, space="PSUM") as psum_pool,
        ):
            # PE p-state warm-up: the tensor clock ramps 0.65->1.2->2.4GHz
            # with ~3us of sustained activity. Burn the dead DMA-latency
            # window (first weights land ~9.5us after the ~7us framework
            # preamble) on dummy matmuls over a zeroed tile so the first real
            # matmuls run closer to full clock.
            warm_sb = bias_pool.tile([P, 256], bf16)
            nc.vector.memset(warm_sb[:], 0.0)
            warm_ps = psum_pool.tile([P, 256], f32, tag="ps")
            for i in range(5):
                nc.tensor.matmul(
                    warm_ps[:],
                    lhsT=warm_sb[:, :P],
                    rhs=warm_sb[:],
                    start=(i == 0),
                    stop=(i == 4),
                )

            xt1_sb = xt_pool.tile([P, KD * C1], bf16)
            xt2_sb = xt_pool.tile([P, KD * C2], bf16)
            # ALL inputs ride the single Sync HWDGE queue, ordered by first
            # consumption: a FIFO queue drains at full aggregate DMA bandwidth
            # and in-order completion is exact prioritization. (Splitting the
            # early loads across gpsimd/scalar queues let the deep w1a
            # prefetch starve the critical xt1 slices.)
            w1_first = w1_pool.tile([P, D], bf16, tag="w1")
            nc.sync.dma_start(w1_first[:], w1a_d[0])
            nc.sync.dma_start(xt1_sb[:, :C1], xt1_d[:, :C1])
            nc.sync.dma_start(xt1_sb[:, C1 : 4 * C1], xt1_d[:, C1 : 4 * C1])
            nc.sync.dma_start(xt1_sb[:, 4 * C1 :], xt1_d[:, 4 * C1 :])
            b1a_sb = bias_pool.tile([P, MH], f32)
            nc.sync.dma_start(b1a_sb[:], b1a_d[:])
            b1b_sb = bias_pool.tile([P, MH], f32)
            nc.sync.dma_start(b1b_sb[:], b1b_d[:])

            ht1_sb = ht_pool.tile([P, MH * C1], bf16)
            ht2_sb = ht_pool.tile([P, MH * C2], bf16)

            def layer1(m, w1_sb, C, chunks, xt_sb, ht_sb, b1_sb):
                t0 = 0
                for tn in chunks:
                    ps = psum_pool.tile([P, 512], f32, tag="ps")
                    for k in range(KD):
                        nc.tensor.matmul(
                            ps[:, :tn],
                            lhsT=w1_sb[:, k * P : (k + 1) * P],
                            rhs=xt_sb[:, k * C + t0 : k * C + t0 + tn],
                            start=(k == 0),
                            stop=(k == KD - 1),
                        )
                    nc.scalar.activation(
                        ht_sb[:, m * C + t0 : m * C + t0 + tn],
                        ps[:, :tn],
                        AF.Relu,
                        bias=b1_sb[:, m : m + 1],
                    )
                    t0 += tn

            def layer2(m2, w2_sb, C, chunks, ht_sb, yt_d, last=False):
                t0 = 0
                for tn in chunks:
                    ps2 = psum_pool.tile([P, 512], f32, tag="ps")
                    for k2 in range(MH):
                        nc.tensor.matmul(
                            ps2[:, :tn],
                            lhsT=w2_sb[:, k2 * P : (k2 + 1) * P],
                            rhs=ht_sb[:, k2 * C + t0 : k2 * C + t0 + tn],
                            start=(k2 == 0),
                            stop=(k2 == MH - 1),
                        )
                    yt_sb = y_pool.tile([P, 512], bf16, tag="yt")
                    # partial sum over this core's DFF half; b2 added on host
                    if last and t0 + tn == C:
                        # kernel-final block is on the critical path after the
                        # last matmul: split activation+store in halves across
                        # the two HWDGE queues (sync + scalar) so the two
                        # store latencies overlap; gpsimd SWDGE is ~1us slower
                        h1 = tn // 2
                        nc.scalar.activation(yt_sb[:, :h1], ps2[:, :h1], AF.Identity)
                        nc.sync.dma_start(yt_d[m2][:, t0 : t0 + h1], yt_sb[:, :h1])
                        nc.scalar.activation(yt_sb[:, h1:tn], ps2[:, h1:tn], AF.Identity)
                        nc.scalar.dma_start(
                            yt_d[m2][:, t0 + h1 : t0 + tn], yt_sb[:, h1:tn]
                        )
                    else:
                        nc.scalar.activation(yt_sb[:, :tn], ps2[:, :tn], AF.Identity)
                        nc.gpsimd.dma_start(yt_d[m2][:, t0 : t0 + tn], yt_sb[:, :tn])
                    t0 += tn

            for m in range(MH):
                if m == 0:
                    w1_sb = w1_first
                else:
                    w1_sb = w1_pool.tile([P, D], bf16, tag="w1")
                    nc.sync.dma_start(w1_sb[:], w1a_d[m])
                if m == 5:
                    # xt2 slots into the Sync stream here: early enough to
                    # land ~25us before L1b consumes it, late enough that it
                    # never delays a w1a block past its consumption time
                    nc.sync.dma_start(xt2_sb[:, : 4 * C2], xt2_d[:, : 4 * C2])
                    nc.sync.dma_start(xt2_sb[:, 4 * C2 :], xt2_d[:, 4 * C2 :])
                layer1(m, w1_sb, C1, chunks1, xt1_sb, ht1_sb, b1a_sb)
            for m in range(MH):
                w1_sb = w1_pool.tile([P, D], bf16, tag="w1")
                nc.sync.dma_start(w1_sb[:], w1b_d[m])
                layer1(m, w1_sb, C2, chunks2, xt2_sb, ht2_sb, b1b_sb)

            for m2 in range(KD):
                w2_sb = w2_pool.tile([P, MH * P], bf16, tag="w2")
                nc.sync.dma_start(w2_sb[:], w2a_d[m2])
                layer2(m2, w2_sb, C1, chunks1, ht1_sb, yt1_d)
            for m2 in range(KD):
                w2_sb = w2_pool.tile([P, MH * P], bf16, tag="w2")
                nc.sync.dma_start(w2_sb[:], w2b_d[m2])
                layer2(m2, w2_sb, C2, chunks2, ht2_sb, yt2_d, last=(m2 == KD - 1))

    nc.compile()
    return nc


MQ = (DFF // 4) // P  # 8 dff-quarter blocks (TP4 layer1 out / layer2 contraction)


def _build_program_tp4(caps):
    """TP4: 4 cores per expert-quad; core holds a DFF/4 slice of 4 experts.

    Near-perfect load balance: per-core rows = 128 * sum(caps) with caps the
    slotwise maxima over the two quads (2088 for this gate vs 2160 for TP2's
    pair scheme) -> ~111.4us ideal row stream vs 115.2. Host sums 4 partials.
    """
    import concourse.mybir as mybir
    import concourse.tile as tile
    from concourse import bacc

    f32 = mybir.dt.float32
    bf16 = mybir.dt.bfloat16
    AF = mybir.ActivationFunctionType

    S = list(caps)
    chunks = [_chunk_sizes(c) for c in S]

    nc = bacc.Bacc("TRN2", target_bir_lowering=False, debug=False, num_devices=E)

    xt_d = [
        nc.dram_tensor(f"xt{s}", [P, KD * S[s]], bf16, kind="ExternalInput").ap()
        for s in range(4)
    ]
    w1q_d = nc.dram_tensor("w1q", [4 * MQ, P, D], bf16, kind="ExternalInput").ap()
    b1q_d = nc.dram_tensor("b1q", [P, 4 * MQ], f32, kind="ExternalInput").ap()
    w2q_d = nc.dram_tensor("w2q", [4 * KD, P, MQ * P], bf16, kind="ExternalInput").ap()
    yt_d = [
        nc.dram_tensor(f"yt{s}", [KD, P, S[s]], bf16, kind="ExternalOutput").ap()
        for s in range(4)
    ]

    with tile.TileContext(nc) as tc:
        with (
            tc.tile_pool(name="xt_pool", bufs=1) as xt_pool,
            tc.tile_pool(name="ht_pool", bufs=1) as ht_pool,
            tc.tile_pool(name="w1_pool", bufs=20) as w1_pool,
            tc.tile_pool(name="w2_pool", bufs=12) as w2_pool,
            tc.tile_pool(name="y_pool", bufs=4) as y_pool,
            tc.tile_pool(name="bias_pool", bufs=1) as bias_pool,
            tc.tile_pool(name="psum", bufs=8, space="PSUM") as psum_pool,
        ):
            # PE p-state warm-up AND idle-bridge: the tensor clock ramps
            # 0.65->1.2->2.4GHz over ~3us of sustained activity and the ramp
            # resets on idle gaps. Dummy matmuls keep the PE busy from the end
            # of the framework preamble (~7us) until xt0 is fully resident
            # (~12us), so the real stream starts at full clock with no gap.
            warm_sb = bias_pool.tile([P, 256], bf16)
            nc.vector.memset(warm_sb[:], 0.0)
            warm_ps = psum_pool.tile([P, 256], f32, tag="ps")
            for i in range(28):
                nc.tensor.matmul(
                    warm_ps[:],
                    lhsT=warm_sb[:, :P],
                    rhs=warm_sb[:],
                    start=(i == 0),
                    stop=(i == 27),
                )

            xt_sb = [
                xt_pool.tile([P, KD * S[s]], bf16, name=f"xt_sb{s}") for s in range(4)
            ]
            ht_sb = [
                ht_pool.tile([P, MQ * S[s]], bf16, name=f"ht_sb{s}") for s in range(4)
            ]

            w1_first = w1_pool.tile([P, D], bf16, tag="w1")
            nc.sync.dma_start(w1_first[:], w1q_d[0])
            # xt0 split across both HWDGE queues: arrival of the full slot-0
            # token block is what gates the start of the real stream
            nc.sync.dma_start(xt_sb[0][:, : 2 * S[0]], xt_d[0][:, : 2 * S[0]])
            nc.scalar.dma_start(xt_sb[0][:, 4 * S[0] :], xt_d[0][:, 4 * S[0] :])
            nc.sync.dma_start(
                xt_sb[0][:, 2 * S[0] : 4 * S[0]], xt_d[0][:, 2 * S[0] : 4 * S[0]]
            )
            b1q_sb = bias_pool.tile([P, 4 * MQ], f32)
            nc.sync.dma_start(b1q_sb[:], b1q_d[:])

            def l1_chunk(e, m, w1_sb, s, rhs_fn, tn, t0):
                C = S[s]
                ps = psum_pool.tile([P, 512], f32, tag="ps")
                for k in range(KD):
                    nc.tensor.matmul(
                        ps[:, :tn],
                        lhsT=w1_sb[:, k * P : (k + 1) * P],
                        rhs=rhs_fn(k),
                        start=(k == 0),
                        stop=(k == KD - 1),
                    )
                nc.scalar.activation(
                    ht_sb[s][:, m * C + t0 : m * C + t0 + tn],
                    ps[:, :tn],
                    AF.Relu,
                    bias=b1q_sb[:, e * MQ + m : e * MQ + m + 1],
                )

            def layer2(e, m2, w2_sb, s, last=False):
                C, t0 = S[s], 0
                for tn in chunks[s]:
                    ps2 = psum_pool.tile([P, 512], f32, tag="ps")
                    for k2 in range(MQ):
                        nc.tensor.matmul(
                            ps2[:, :tn],
                            lhsT=w2_sb[:, k2 * P : (k2 + 1) * P],
                            rhs=ht_sb[s][:, k2 * C + t0 : k2 * C + t0 + tn],
                            start=(k2 == 0),
                            stop=(k2 == MQ - 1),
                        )
                    yt_sb = y_pool.tile([P, 512], bf16, tag="yt")
                    if last and t0 + tn == C:
                        h1 = tn // 2
                        nc.scalar.activation(yt_sb[:, :h1], ps2[:, :h1], AF.Identity)
                        nc.sync.dma_start(yt_d[s][m2][:, t0 : t0 + h1], yt_sb[:, :h1])
                        nc.scalar.activation(yt_sb[:, h1:tn], ps2[:, h1:tn], AF.Identity)
                        nc.scalar.dma_start(
                            yt_d[s][m2][:, t0 + h1 : t0 + tn], yt_sb[:, h1:tn]
                        )
                    else:
                        nc.scalar.activation(yt_sb[:, :tn], ps2[:, :tn], AF.Identity)
                        nc.gpsimd.dma_start(
                            yt_d[s][m2][:, t0 : t0 + tn], yt_sb[:, :tn]
                        )
                    t0 += tn

            def load_xt(s):
                nc.sync.dma_start(xt_sb[s][:, : 4 * S[s]], xt_d[s][:, : 4 * S[s]])
                nc.sync.dma_start(xt_sb[s][:, 4 * S[s] :], xt_d[s][:, 4 * S[s] :])

            # xt_s streams into the Sync queue a full expert-phase early,
            # spaced >=4 weight blocks apart so w1 never lands late
            xt_insert = {(0, 4): 1, (1, 0): 2, (1, 4): 3}
            for e in range(4):
                for m in range(MQ):
                    if e == 0 and m == 0:
                        w1_sb = w1_first
                    else:
                        w1_sb = w1_pool.tile([P, D], bf16, tag="w1")
                        nc.sync.dma_start(w1_sb[:], w1q_d[e * MQ + m])
                    s = xt_insert.get((e, m))
                    if s is not None:
                        load_xt(s)
                    C, te = S[e], 0
                    for tn in chunks[e]:
                        xe = xt_sb[e]
                        l1_chunk(
                            e, m, w1_sb, e,
                            lambda k, xe=xe, C=C, te=te, tn=tn: xe[
                                :, k * C + te : k * C + te + tn
                            ],
                            tn, te,
                        )
                        te += tn

            for e in range(4):
                for m2 in range(KD):
                    w2_sb = w2_pool.tile([P, MQ * P], bf16, tag="w2")
                    nc.sync.dma_start(w2_sb[:], w2q_d[e * KD + m2])
                    layer2(e, m2, w2_sb, e, last=(e == 3 and m2 == KD - 1))

    nc.compile()
    return nc


def _arrange_w1_quarter(w1_e, h):
    """w1 quarter: [D, 1024] -> [MQ, P, D] with [m,p,k*128+j] = w1[k*128+p, off+m*128+j]."""
    q = w1_e[:, h * (MQ * P) : (h + 1) * (MQ * P)].astype(BF16)
    return np.ascontiguousarray(
        q.reshape(KD, P, MQ, P).transpose(2, 1, 0, 3).reshape(MQ, P, D)
    )


def _arrange_w2_quarter(w2_e, h):
    """w2 quarter: [1024, D] -> [KD, P, MQ*P] with [m2,p,k2*128+j] = w2[off+k2*128+p, m2*128+j]."""
    q = w2_e[h * (MQ * P) : (h + 1) * (MQ * P), :].astype(BF16)
    return np.ascontiguousarray(
        q.reshape(MQ, P, KD, P).transpose(2, 1, 0, 3).reshape(KD, P, MQ * P)
    )


def _run_pass_tp4(x, w1, b1, w2, b2, idx, out):
    from concourse.bass_utils import run_bass_kernel_spmd

    global LAST_RESULT

    counts = np.array([len(i) for i in idx])
    order = np.argsort(-counts, kind="stable")
    # slot j holds the (2j)-th and (2j+1)-th largest experts, one per quad:
    # slot capacity = the larger of the two, and sum(caps) is minimal
    groups = [[int(order[2 * j + g]) for j in range(4)] for g in range(2)]
    caps = tuple(_pad_cap(max(counts[order[2 * j]], counts[order[2 * j + 1]]))
                 for j in range(4))

    key = ("tp4",) + caps
    if key not in _PROGRAM_CACHE:
        _PROGRAM_CACHE[key] = _build_program_tp4(caps)
    nc = _PROGRAM_CACHE[key]

    in_maps = []
    for c in range(E):
        g, h = divmod(c, 4)
        exps = groups[g]
        m = {
            "w1q": np.concatenate([_arrange_w1_quarter(w1[e], h) for e in exps]),
            "b1q": np.ascontiguousarray(
                np.concatenate(
                    [
                        b1[e][h * (MQ * P) : (h + 1) * (MQ * P)].reshape(MQ, P).T
                        for e in exps
                    ],
                    axis=1,
                )
            ),
            "w2q": np.concatenate([_arrange_w2_quarter(w2[e], h) for e in exps]),
        }
        for s, e in enumerate(exps):
            m[f"xt{s}"] = _arrange_tokens(x[idx[e]], caps[s])
        in_maps.append(m)

    res = run_bass_kernel_spmd(
        nc,
        in_maps,
        core_ids=list(range(E)),
        trace=TRACE,
        **({"trace_cores": TRACE_CORES} if TRACE_CORES else {}),
    )
    LAST_RESULT = res

    for g in range(2):
        for s, e in enumerate(groups[g]):
            n = len(idx[e])
            if n == 0:
                continue
            yt = sum(
                res.results[4 * g + h][f"yt{s}"].astype(np.float32) for h in range(4)
            )
            ye = yt.transpose(2, 0, 1).reshape(-1, D)
            out[idx[e]] = ye[:n] + b2[e]


def _arrange_tokens(x_e, C):
    """[n, D] tokens -> xt[p, k*C + c] = x_e[c, k*128 + p], zero-padded, bf16."""
    xe = np.zeros((C, D), BF16)
    xe[: len(x_e)] = x_e
    return np.ascontiguousarray(
        xe.T.reshape(KD, P, C).transpose(1, 0, 2).reshape(P, KD * C)
    )


def _arrange_w1_half(w1_e, h):
    """w1 half: [D, 2048] -> [MH, P, D] with [m, p, k*128+j] = w1[k*128+p, off+m*128+j]."""
    half = w1_e[:, h * (MH * P) : (h + 1) * (MH * P)].astype(BF16)
    return np.ascontiguousarray(
        half.reshape(KD, P, MH, P).transpose(2, 1, 0, 3).reshape(MH, P, D)
    )


def _arrange_w2_half(w2_e, h):
    """w2 half: [2048, D] -> [KD, P, 2048] with [m2, p, k2*128+j] = w2[off+k2*128+p, m2*128+j]."""
    half = w2_e[h * (MH * P) : (h + 1) * (MH * P), :].astype(BF16)
    return np.ascontiguousarray(
        half.reshape(MH, P, KD, P).transpose(2, 1, 0, 3).reshape(KD, P, MH * P)
    )


def kernel(x, gate_w, gate_b, w1, b1, w2, b2):
    from concourse.bass_utils import run_bass_kernel_spmd

    global LAST_RESULT

    x = np.ascontiguousarray(np.asarray(x, dtype=np.float32))
    gate_w = np.asarray(gate_w, dtype=np.float32)
    gate_b = np.asarray(gate_b, dtype=np.float32)
    w1 = np.asarray(w1, dtype=np.float32)
    b1 = np.asarray(b1, dtype=np.float32)
    w2 = np.asarray(w2, dtype=np.float32)
    b2 = np.asarray(b2, dtype=np.float32)
    n_tok = x.shape[0]

    # host gate + top-1 routing (fp64: exact argmax, see module docstring)
    logits = x.astype(np.float64) @ gate_w.astype(np.float64) + gate_b.astype(
        np.float64
    )
    assign = np.argmax(logits, axis=-1)
    idx_full = [np.nonzero(assign == e)[0] for e in range(E)]

    # Defensive slabbing: if routing were pathologically imbalanced, process
    # tokens in passes so per-expert capacity stays within SBUF limits. With
    # the benchmark's near-uniform gate this is a single pass.
    slab = 960
    n_pass = max(1, -(-max(len(i) for i in idx_full) // slab))
    out = np.zeros((n_tok, D), np.float32)
    for ps in range(n_pass):
        idx = [i[ps * slab : (ps + 1) * slab] for i in idx_full]
        _run_pass(x, w1, b1, w2, b2, idx, out)
    return out


USE_TP4 = True


def _run_pass(x, w1, b1, w2, b2, idx, out):
    from concourse.bass_utils import run_bass_kernel_spmd

    global LAST_RESULT

    if USE_TP4:
        return _run_pass_tp4(x, w1, b1, w2, b2, idx, out)

    counts = np.array([len(i) for i in idx])

    # pair experts large-with-small to balance per-core token load
    order = np.argsort(-counts, kind="stable")
    pairs = [(int(order[p]), int(order[E - 1 - p])) for p in range(E // 2)]
    C1 = _pad_cap(max(counts[a] for a, _ in pairs))
    C2 = _pad_cap(max(counts[b] for _, b in pairs))

    key = (C1, C2)
    if key not in _PROGRAM_CACHE:
        _PROGRAM_CACHE[key] = _build_program_tp2(C1, C2)
    nc = _PROGRAM_CACHE[key]

    in_maps = []
    for c in range(E):
        p, h = divmod(c, 2)
        ea, eb = pairs[p]
        in_maps.append(
            {
                "xt1": _arrange_tokens(x[idx[ea]], C1),
                "xt2": _arrange_tokens(x[idx[eb]], C2),
                "w1a": _arrange_w1_half(w1[ea], h),
                "w1b": _arrange_w1_half(w1[eb], h),
                "b1a": np.ascontiguousarray(
                    b1[ea][h * (MH * P) : (h + 1) * (MH * P)].reshape(MH, P).T
                ),
                "b1b": np.ascontiguousarray(
                    b1[eb][h * (MH * P) : (h + 1) * (MH * P)].reshape(MH, P).T
                ),
                "w2a": _arrange_w2_half(w2[ea], h),
                "w2b": _arrange_w2_half(w2[eb], h),
            }
        )

    res = run_bass_kernel_spmd(
        nc,
        in_maps,
        core_ids=list(range(E)),
        trace=TRACE,
        **({"trace_cores": TRACE_CORES} if TRACE_CORES else {}),
    )
    LAST_RESULT = res

    for p in range(E // 2):
        ea, eb = pairs[p]
        for slot, e in (("yt1", ea), ("yt2", eb)):
            n = len(idx[e])
            if n == 0:
                continue
            # sum the two DFF-half partials (bf16 on the wire, fp32 combine),
            # restore [tokens, D], add b2
            yt = res.results[2 * p][slot].astype(np.float32) + res.results[2 * p + 1][
                slot
            ].astype(np.float32)
            ye = yt.transpose(2, 0, 1).reshape(-1, D)
            out[idx[e]] = ye[:n] + b2[e]

